# revision 5
# baseline (speedup 1.0000x reference)
"""Trainium2 Bass kernel for nn_AdaptiveHyperNN (gnn_message_passing).

Math identical to v1 (edge MLPs are linear before aggregation; the network
folds host-side to three projection vectors over the gathered embeddings):

  out[u*N+v] = sigmoid(feat_u@gA + feat_v@gB + sum_w feat_w@aABn + c0b)

v5 schedule (same gather + PE-transpose front end as v1, rebalanced):

- t row computed FIRST so the DVE reduce (the long C chain) overlaps the
  p matmuls; q accumulates STRAIGHT into the logits PSUM via two matmuls
  against host-broadcast gB tiles (no PSUM->SBUF cast, no ones-broadcast,
  no cross-engine hop on the q path)
- per-graph constant c0b is folded into the p_sb copy as a tensor_scalar
  add against a host-staged f32 column (no rank-1 injects for it)
- C injected into the p column right before the ones x q broadcast (the
  two rank-1s share the ones LDWEIGHTS)
- ones row staged via DMA (no DVE memsets); ACT warm gates on the gather
- both output DMAs issue from SP (rows 0-63 after h0, 64-127 after h1);
  SP's end-of-program wrap is the cheapest of the engines
- bass start/end barriers, end drains and const-AP memsets stripped
  post-build (ordering flows through this kernel's own semaphores; the
  NEFF epilogue drains the rings)
"""

import numpy as np
import ml_dtypes

import concourse.bacc as bacc
import concourse.bass as bass
import concourse.mybir as mybir

P = 128
D = 256
N = 128
B = 8
V = 10000
F32 = mybir.dt.float32
BF16 = mybir.dt.bfloat16
I32 = mybir.dt.int32


def build_nc():
    nc = bacc.Bacc("TRN2", target_bir_lowering=False)
    AO = mybir.AluOpType
    TSF = mybir.ActivationFunctionType

    inv = nc.dram_tensor("invoked", [N, 1], I32, kind="ExternalInput")
    emb = nc.dram_tensor("emb", [V, D], BF16, kind="ExternalInput")
    gwD = nc.dram_tensor("gw", [P, 8], BF16, kind="ExternalInput")
    rwD = nc.dram_tensor("rw", [1, P], BF16, kind="ExternalInput")
    gbD = nc.dram_tensor("gb", [P, 2 * P], BF16, kind="ExternalInput")
    cwD = nc.dram_tensor("cw", [P, 1], F32, kind="ExternalInput")
    identd = nc.dram_tensor("identd", [P, P], BF16, kind="ExternalInput")
    out = nc.dram_tensor("out", [N, N], BF16, kind="ExternalOutput")

    sb = nc.alloc_sbuf_tensor
    inv_t = sb("inv_t", [P, 1], I32)
    ident = sb("ident", [P, P], BF16)
    feat = sb("feat", [P, D], BF16)
    gw_sb = sb("gw_sb", [P, 8], BF16)
    rw = sb("rw_t", [1, P], BF16)      # ones row
    gb = sb("gb_t", [P, 2 * P], BF16)  # gB chunks broadcast along M
    cw = sb("cw_t", [P, 1], F32)       # c0b replicated per partition
    featTs = sb("featTs", [P, D], BF16)
    featT = [featTs[:, 0:P], featTs[:, P : 2 * P]]
    cbf = sb("cbf", [1, 1], BF16)
    p_sb = sb("p_sb", [P, 1], F32)
    osb = sb("osb", [P, P], BF16)
    warm = sb("warm", [1, 1], F32)

    pp = nc.alloc_psum_tensor
    PT0 = pp("PT0", [P, P], BF16)
    PT1 = pp("PT1", [P, P], BF16)
    PTr = pp("PTr", [1, P], F32)
    PCOL = pp("PCOL", [P, 1], F32)
    PBC = pp("PBC", [P, P], F32)

    with (
        nc.Block() as block,
        nc.semaphore("dI") as dI,
        nc.semaphore("dW") as dW,
        nc.semaphore("dG") as dG,
        nc.semaphore("dOUT") as dOUT,
        nc.semaphore("dID") as dID,
        nc.semaphore("sT") as sT,
        nc.semaphore("sV") as sV,
        nc.semaphore("sA") as sA,
    ):

        @block.gpsimd
        def _(gpsimd):
            gpsimd.wait_ge(dI, 16)
            gpsimd.indirect_dma_start(
                out=feat[:],
                out_offset=None,
                in_=emb[:, :],
                in_offset=bass.IndirectOffsetOnAxis(ap=inv_t[:, :1], axis=0),
            ).then_inc(dG, 16)

        @block.sync
        def _(sync):
            sync.dma_start(out=inv_t[:], in_=inv[:, :], single_packet=True).then_inc(dI, 16)
            sync.dma_start(out=ident[:], in_=identd[:, :]).then_inc(dID, 16)
            sync.dma_start(out=rw[:, 0:P], in_=rwD[:, :]).then_inc(dW, 16)
            sync.dma_start(out=cw[:], in_=cwD[:, :]).then_inc(dW, 16)
            sync.wait_ge(sA, 1)
            sync.dma_start(out=out[0:64, :], in_=osb[0:64, :]).then_inc(dOUT, 16)

        @block.scalar
        def _(scalar):
            scalar.dma_start(out=gw_sb[:], in_=gwD[:, :]).then_inc(dW, 16)
            scalar.dma_start(out=gb[:], in_=gbD[:, :]).then_inc(dW, 16)
            scalar.wait_ge(dG, 16)
            nc.scalar.activation(
                out=warm[:], in_=gw_sb[0:1, 7:8], func=TSF.Sigmoid,
                bias=gw_sb[0:1, 7:8],
            )
            scalar.wait_ge(sT, 5)
            scalar.wait_ge(sV, 4)
            nc.scalar.activation(
                out=osb[0:64, :], in_=PBC[0:64, :], func=TSF.Sigmoid,
                bias=p_sb[0:64, 0:1],
            ).then_inc(sA, 1)
            nc.scalar.activation(
                out=osb[64:128, :], in_=PBC[64:128, :], func=TSF.Sigmoid,
                bias=p_sb[64:128, 0:1],
            ).then_inc(sA, 1)
            scalar.wait_ge(sA, 2)
            scalar.dma_start(out=out[64:128, :], in_=osb[64:128, :]).then_inc(dOUT, 16)

        @block.tensor
        def _(tensor):
            mm = nc.tensor.matmul
            tensor.wait_ge(dID, 16)
            tensor.wait_ge(dG, 16)
            nc.tensor.transpose(out=PT0[:], in_=feat[:, 0:P], identity=ident[:]).then_inc(sT, 1)
            nc.tensor.transpose(out=PT1[:], in_=feat[:, P : 2 * P], identity=ident[:]).then_inc(sT, 1)
            tensor.wait_ge(dW, 64)
            tensor.wait_ge(sV, 1)
            # t row first: the reduce -> C chain is the long pole
            mm(out=PTr[:, :], lhsT=gw_sb[:, 2:3], rhs=featT[0], start=True, stop=False)
            tensor.wait_ge(sV, 2)
            mm(out=PTr[:, :], lhsT=gw_sb[:, 5:6], rhs=featT[1], start=False, stop=True).then_inc(sT, 1)
            # p column
            mm(out=PCOL[:, :], lhsT=featT[0], rhs=gw_sb[:, 0:1], start=True, stop=False)
            mm(out=PCOL[:, :], lhsT=featT[1], rhs=gw_sb[:, 3:4], start=False, stop=False)
            # q straight into PBC via host-broadcast gB (no cast, no hops)
            mm(out=PBC[:, :], lhsT=gb[:, 0:P], rhs=featT[0], start=True, stop=False)
            # C into the p column while cbf arrives
            tensor.wait_ge(sV, 3)
            mm(out=PCOL[:, :], lhsT=rw[0:1, 0:P], rhs=cbf[:, :], start=False, stop=True).then_inc(sT, 1)
            mm(out=PBC[:, :], lhsT=gb[:, P : 2 * P], rhs=featT[1], start=False, stop=True).then_inc(sT, 1)

        @block.vector
        def _(vector):
            vector.wait_ge(sT, 1)
            nc.vector.tensor_copy(out=featT[0], in_=PT0[:]).then_inc(sV, 1)
            vector.wait_ge(sT, 2)
            nc.vector.tensor_copy(out=featT[1], in_=PT1[:]).then_inc(sV, 1)
            vector.wait_ge(sT, 3)
            with nc.allow_low_precision("C accumulates ~128 tiny terms; bf16 is plenty"):
                nc.vector.tensor_reduce(
                    out=cbf[:, :], in_=PTr[0:1, :], axis=mybir.AxisListType.X, op=AO.add
                ).then_inc(sV, 1)
            vector.wait_ge(sT, 4)
            nc.vector.tensor_scalar(
                out=p_sb[:, :], in0=PCOL[:, :], scalar1=cw[:, 0:1], scalar2=None,
                op0=AO.add,
            ).then_inc(sV, 1)

    import concourse.mybir as _mb
    for bb in nc.m.functions[0].blocks:
        if bb.name == "main":
            bb.instructions = [
                i for i in bb.instructions
                if not i.name.startswith("barrier_")
                and not isinstance(i, _mb.InstDrain)
                and not isinstance(i, _mb.InstMemset)
            ]
        elif bb.name.endswith("_end"):
            bb.instructions = [
                i for i in bb.instructions
                if not i.name.startswith("barrier_")
                and not isinstance(i, _mb.InstDrain)
            ]
    nc.compile()
    return nc


TRACE = False
LAST_RESULTS = None
_NC_CACHE = {}


def kernel(Xs, api_embeds, W1, b1, W2, b2, W3, b3, W4, b4, invoked):
    global LAST_RESULTS
    from concourse.bass_utils import run_bass_kernel_spmd

    if "nc" not in _NC_CACHE:
        _NC_CACHE["nc"] = build_nc()
    nc = _NC_CACHE["nc"]

    Xs = np.asarray(Xs, dtype=np.float64)
    emb = np.asarray(api_embeds, dtype=np.float32)
    W1 = np.asarray(W1, dtype=np.float64)
    W2 = np.asarray(W2, dtype=np.float64)
    W3 = np.asarray(W3, dtype=np.float64)
    W4 = np.asarray(W4, dtype=np.float64).reshape(2 * D)
    b1 = np.asarray(b1, dtype=np.float64).reshape(D)
    b2 = np.asarray(b2, dtype=np.float64).reshape(D)
    b3 = np.asarray(b3, dtype=np.float64).reshape(D)
    b4 = np.asarray(b4, dtype=np.float64).reshape(1)
    invoked = np.asarray(invoked, dtype=np.int32)

    W1a, W1b = W1[:D], W1[D:]
    W2a, W2b = W2[:D], W2[D:]
    W3a, W3b = W3[:D], W3[D:]
    W4a, W4b = W4[:D], W4[D:]
    M = W2a + W1b @ W2b
    A = W1a @ W2b
    bh = b1 @ W2b + b2
    wA = W3a @ W4a
    wB = W3b @ W4a
    gA = M @ wA
    gB = M @ wB
    aABn = (A @ (wA + wB)) / N
    c0 = bh @ (wA + wB) + b3 @ W4a + b4[0]

    emb_bf = np.ascontiguousarray(emb.astype(ml_dtypes.bfloat16))
    identv = np.ascontiguousarray(np.eye(P, dtype=ml_dtypes.bfloat16))
    rwv = np.ones((1, P), dtype=ml_dtypes.bfloat16)
    gbv = np.zeros((P, 2 * P), dtype=ml_dtypes.bfloat16)
    gbv[:, 0:P] = np.asarray(gB[0:P], dtype=np.float32)[:, None]
    gbv[:, P : 2 * P] = np.asarray(gB[P : 2 * P], dtype=np.float32)[:, None]

    gwv0 = np.zeros((P, 8), dtype=ml_dtypes.bfloat16)
    for kt in range(2):
        gwv0[:, 3 * kt + 0] = gA[kt * P : (kt + 1) * P]
        gwv0[:, 3 * kt + 1] = gB[kt * P : (kt + 1) * P]
        gwv0[:, 3 * kt + 2] = aABn[kt * P : (kt + 1) * P]

    in_maps = []
    for b in range(B):
        c0b = c0 + Xs[b] @ W4b
        cwv = np.full((P, 1), c0b, dtype=np.float32)
        in_maps.append(
            {
                "invoked": np.ascontiguousarray(invoked[b].reshape(N, 1)),
                "emb": emb_bf,
                "gw": gwv0,
                "rw": rwv,
                "gb": gbv,
                "cw": cwv,
                "identd": identv,
            }
        )

    res = run_bass_kernel_spmd(nc, in_maps, core_ids=list(range(B)), trace=TRACE)
    LAST_RESULTS = res
    return np.stack(
        [
            np.asarray(res.results[i]["out"], dtype=np.float32).reshape(N * N, 1)
            for i in range(B)
        ],
        axis=0,
    )
